# revision 4
# baseline (speedup 1.0000x reference)
"""Trainium2 Bass kernel for CrossDecoder kNN-mining margin loss.

Math: reference mines, per query q (both columns of train_ill), the k+1
nearest rows of X (rows = concat of both manifolds, dim 512) and uses the
*distances* from q to its own k nearest neighbours (self excluded) in a
margin loss.  Ranking and values only need, per query row, the top-(k+1)
smallest of  dist(q, j) = |q|^2 + |y_j|^2 - 2 q.y_j ; since |q|^2 is
row-constant we rank by  score(q,j) = 2 q.y_j - |y_j|^2  (descending) and
recover dist = |q|^2 - score on the host.

Device work (SPMD over 8 cores, candidate axis sharded 30000 -> 3750/core):
  - scores for a [128-query, 469-candidate] tile = 4 accumulated K=128
    fp32r matmuls (queries pre-scaled by 2) plus one K=1 matmul with
    lhsT = -1 row and rhs = |y|^2 row (folds the -|y|^2 bias into PSUM).
  - nc.vector.max (top-8, descending) per chunk straight off PSUM.
Each core emits, per query, 8 chunks x top-8 = 64 candidate scores.
Host merges 8 cores x 64 = 512 candidates/row -> exact top-(k+1) w.p. 1
(would only fail if >8 of the true top-11 landed in one 469-wide chunk).
"""

import os
import numpy as np

M_, N_, D_, T_ = 2, 30000, 256, 3000
NCORES = 8
NSHARD = N_ // NCORES          # 3750
FCH = 470                      # candidate chunk width (>=256 keeps fp32r at full
                               # rate; must be EVEN: fp32r ISA requires even
                               # innermost free count on moving operand and dst)
NFC = 8                        # chunks per shard
NPAD = FCH * NFC               # 3760
KD = M_ * D_                   # 512 contraction dim
KCH = KD // 128                # 4 K-chunks
QT = 128                       # queries per tile (PSUM partition dim)
TPAD = 3072                    # per-set padded query count (24 tiles)
NQ = 2 * TPAD                  # both query sets
NQT = NQ // QT                 # 48 query tiles
QBLK = 4                       # query tiles per DMA block
NBLK = NQT // QBLK             # 12
PAD_SQY = 1.0e30               # pad candidates rank last

_cache = {}


def _build_program():
    import concourse.bass as bass
    import concourse.tile as tile
    from concourse import bacc, mybir

    dt = mybir.dt
    nc = bacc.Bacc(
        "TRN2", target_bir_lowering=False, debug=False, num_devices=NCORES
    )

    xq_d = nc.dram_tensor("xq", [KCH, 128, NQ], dt.float32r, kind="ExternalInput")
    xs_d = nc.dram_tensor("xs", [KCH, 128, NPAD], dt.float32r, kind="ExternalInput")
    sqy_d = nc.dram_tensor("sqy", [1, NPAD], dt.float32r, kind="ExternalInput")
    neg1_d = nc.dram_tensor("neg1", [1, QT], dt.float32r, kind="ExternalInput")
    cand_d = nc.dram_tensor("cand", [NBLK, 128, QBLK * 64], dt.float32,
                            kind="ExternalOutput")

    with tile.TileContext(nc) as tc:
        with (
            tc.tile_pool(name="resident", bufs=1) as res_pool,
            tc.tile_pool(name="xq", bufs=2) as xq_pool,
            tc.tile_pool(name="cand", bufs=2) as cand_pool,
            tc.tile_pool(name="psum", bufs=6, space=bass.MemorySpace.PSUM) as psum_pool,
        ):
            xs_sb = [res_pool.tile([128, NPAD], dt.float32r, tag=f"xs{k}", name=f"xs_sb{k}")
                     for k in range(KCH)]
            for k in range(KCH):
                nc.sync.dma_start(out=xs_sb[k][:, :], in_=xs_d[k])
            sqy_sb = res_pool.tile([1, NPAD], dt.float32r, tag="sqy")
            nc.sync.dma_start(out=sqy_sb[:, :], in_=sqy_d[:, :])
            neg1_sb = res_pool.tile([1, QT], dt.float32r, tag="neg1")
            nc.sync.dma_start(out=neg1_sb[:, :], in_=neg1_d[:, :])

            for blk in range(NBLK):
                q0 = blk * QBLK * QT
                xq_sb = [xq_pool.tile([128, QBLK * QT], dt.float32r, tag=f"xq{k}", name=f"xq_sb{k}")
                         for k in range(KCH)]
                for k in range(KCH):
                    nc.sync.dma_start(out=xq_sb[k][:, :],
                                      in_=xq_d[k, :, q0:q0 + QBLK * QT])
                cand_sb = cand_pool.tile([128, QBLK * 64], dt.float32, tag="cand")
                for j in range(QBLK):
                    for f in range(NFC):
                        ps = psum_pool.tile([128, FCH], dt.float32, tag="ps")
                        for k in range(KCH):
                            nc.tensor.matmul(
                                ps[:, :],
                                lhsT=xq_sb[k][:, j * QT:(j + 1) * QT],
                                rhs=xs_sb[k][:, f * FCH:(f + 1) * FCH],
                                start=(k == 0),
                                stop=False,
                            )
                        nc.tensor.matmul(
                            ps[:, :],
                            lhsT=neg1_sb[:, :],
                            rhs=sqy_sb[:, f * FCH:(f + 1) * FCH],
                            start=False,
                            stop=True,
                        )
                        o = j * 64 + f * 8
                        nc.vector.max(cand_sb[:, o:o + 8], ps[:, :])
                nc.sync.dma_start(out=cand_d[blk], in_=cand_sb[:, :])

    nc.compile()
    return nc


def _get_program():
    if "nc" not in _cache:
        _cache["nc"] = _build_program()
    return _cache["nc"]


def _prep_inputs(X, left, right):
    """X: [N, 512] fp32. Returns (shared xq map entries, per-core xs/sqy)."""
    q_idx = np.concatenate([
        right, np.zeros(TPAD - T_, np.int64),
        left, np.zeros(TPAD - T_, np.int64),
    ])
    Xq = (2.0 * X[q_idx]).astype(np.float32)
    Xq[T_:TPAD] = 0.0
    Xq[TPAD + T_:] = 0.0
    xq_in = np.ascontiguousarray(Xq.T.reshape(KCH, 128, NQ))
    neg1 = np.full((1, QT), -1.0, np.float32)

    per_core = []
    for corei in range(NCORES):
        shard = X[corei * NSHARD:(corei + 1) * NSHARD]          # [3750, 512]
        xs = np.zeros((KD, NPAD), np.float32)
        xs[:, :NSHARD] = shard.T
        sqy = np.full((1, NPAD), PAD_SQY, np.float32)
        sqy[0, :NSHARD] = (shard.astype(np.float64) ** 2).sum(1).astype(np.float32)
        per_core.append({
            "xq": xq_in,
            "xs": np.ascontiguousarray(xs.reshape(KCH, 128, NPAD)),
            "sqy": sqy,
            "neg1": neg1,
        })
    return per_core


def _mine_scores(in_maps, trace=False):
    from concourse.bass_utils import run_bass_kernel_spmd

    nc = _get_program()
    try:
        res = run_bass_kernel_spmd(nc, in_maps, list(range(NCORES)), trace=trace)
    except Exception:
        if not trace:
            raise
        res = run_bass_kernel_spmd(nc, in_maps, list(range(NCORES)), trace=False)
    _cache["last_result"] = res
    # per-core cand: [NBLK, 128, QBLK*64] -> [NQ, 64]
    cores = []
    for i in range(NCORES):
        c = res.results[i]["cand"].reshape(NBLK, 128, QBLK, 64)
        cores.append(c.transpose(0, 2, 1, 3).reshape(NQ, 64))
    return np.concatenate(cores, axis=1)                         # [NQ, 512]


def kernel(outlayer, c, train_ill, k):
    k = int(k)
    outlayer = np.asarray(outlayer, np.float32)
    train_ill = np.asarray(train_ill)
    X = np.ascontiguousarray(
        outlayer.transpose(1, 0, 2).reshape(N_, KD)).astype(np.float32)
    left = train_ill[:, 0].astype(np.int64)
    right = train_ill[:, 1].astype(np.int64)

    in_maps = _prep_inputs(X, left, right)
    scores = _mine_scores(in_maps, trace=bool(int(os.environ.get("KNN_TRACE", "0"))))

    # top-(k+1) scores (descending) per query row; row 0 is the self match.
    nkeep = k + 1
    part = np.partition(scores, scores.shape[1] - nkeep, axis=1)[:, -nkeep:]
    top = np.sort(part, axis=1)[:, ::-1]                         # [NQ, k+1]

    X64 = X.astype(np.float64)
    sq = (X64 ** 2).sum(1)                                       # [N]

    s_right = top[:T_]                                           # mining of right idx
    s_left = top[TPAD:TPAD + T_]                                 # mining of left idx

    # B[i, j] = dist(q_i, j-th NN of q_i) = |q_i|^2 - score, self (col 0) dropped
    B2 = sq[right][:, None] - s_right[:, 1:].astype(np.float64)
    B1 = sq[left][:, None] - s_left[:, 1:].astype(np.float64)

    D = ((X64[left] - X64[right]) ** 2).sum(1) + 1.0             # [t]
    L1 = np.maximum(D[:, None] - B1, 0.0)
    L2 = np.maximum(D[:, None] - B2, 0.0)
    loss = (L1.mean() + L2.mean()) / 2.0
    return np.asarray(loss, dtype=np.float32)


# revision 8
# speedup vs baseline: 1.0647x; 1.0647x over previous
"""Trainium2 Bass kernel for CrossDecoder kNN-mining margin loss.

Math: reference mines, per query q (both columns of train_ill), the k+1
nearest rows of X (rows = concat of both manifolds, dim 512) and uses the
*distances* from q to its own k nearest neighbours (self excluded) in a
margin loss.  Ranking and values only need, per query row, the top-(k+1)
smallest of  dist(q, j) = |q|^2 + |y_j|^2 - 2 q.y_j ; since |q|^2 is
row-constant we rank by  score(q,j) = 2 q.y_j - |y_j|^2  (descending) and
recover dist = |q|^2 - score on the host.

Device work (SPMD over 8 cores, candidate axis sharded 30000 -> 3750/core):
  - scores for a [128-query, 469-candidate] tile = 4 accumulated K=128
    fp32r matmuls (queries pre-scaled by 2) plus one K=1 matmul with
    lhsT = -1 row and rhs = |y|^2 row (folds the -|y|^2 bias into PSUM).
  - nc.vector.max (top-8, descending) per chunk straight off PSUM.
Each core emits, per query, 8 chunks x top-8 = 64 candidate scores.
Host merges 8 cores x 64 = 512 candidates/row -> exact top-(k+1) w.p. 1
(would only fail if >8 of the true top-11 landed in one 469-wide chunk).
"""

import os
import numpy as np

M_, N_, D_, T_ = 2, 30000, 256, 3000
NCORES = 8
NSHARD = N_ // NCORES          # 3750
FCH = 470                      # candidate chunk width (>=256 keeps fp32r at full
                               # rate; must be EVEN: fp32r ISA requires even
                               # innermost free count on moving operand and dst)
NFC = 8                        # chunks per shard
NPAD = FCH * NFC               # 3760
KD = M_ * D_                   # 512 contraction dim
KCH = KD // 128                # 4 K-chunks
QT = 128                       # queries per tile (PSUM partition dim)
TPAD = 3072                    # per-set padded query count (24 tiles)
NQ = 2 * TPAD                  # both query sets
NQT = NQ // QT                 # 48 query tiles
QBLK = 4                       # query tiles per DMA block
NBLK = NQT // QBLK             # 12
PAD_SQY = 1.0e30               # pad candidates rank last

_cache = {}


def _build_program():
    import concourse.bass as bass
    import concourse.tile as tile
    from concourse import bacc, mybir

    dt = mybir.dt
    nc = bacc.Bacc(
        "TRN2", target_bir_lowering=False, debug=False, num_devices=NCORES
    )

    xq_d = nc.dram_tensor("xq", [KCH, 128, NQ], dt.float32r, kind="ExternalInput")
    xs_d = nc.dram_tensor("xs", [KCH, 128, NPAD], dt.float32r, kind="ExternalInput")
    # -|y|^2 bias, split hi/lo into bf16 so the bias matmul is a cheap
    # 1-pass bf16 op (fp32r matmuls pay 2x when start=True or K is odd,
    # so the bias matmul opens each PSUM accumulation group instead).
    sqy_d = nc.dram_tensor("sqy", [2, NPAD], dt.bfloat16, kind="ExternalInput")
    neg1_d = nc.dram_tensor("neg1", [2, QT], dt.bfloat16, kind="ExternalInput")
    cand_d = nc.dram_tensor("cand", [NBLK, 128, QBLK * 64], dt.float32,
                            kind="ExternalOutput")

    with tile.TileContext(nc) as tc:
        with (
            tc.tile_pool(name="resident", bufs=1) as res_pool,
            tc.tile_pool(name="xq", bufs=2) as xq_pool,
            tc.tile_pool(name="cand", bufs=2) as cand_pool,
            tc.tile_pool(name="psum", bufs=6, space=bass.MemorySpace.PSUM) as psum_pool,
        ):
            xs_sb = [res_pool.tile([128, NPAD], dt.float32r, tag=f"xs{k}", name=f"xs_sb{k}")
                     for k in range(KCH)]
            for k in range(KCH):
                nc.sync.dma_start(out=xs_sb[k][:, :], in_=xs_d[k])
            sqy_sb = res_pool.tile([2, NPAD], dt.bfloat16, tag="sqy")
            nc.sync.dma_start(out=sqy_sb[:, :], in_=sqy_d[:, :])
            neg1_sb = res_pool.tile([2, QT], dt.bfloat16, tag="neg1")
            nc.sync.dma_start(out=neg1_sb[:, :], in_=neg1_d[:, :])

            for blk in range(NBLK):
                q0 = blk * QBLK * QT
                xq_sb = [xq_pool.tile([128, QBLK * QT], dt.float32r, tag=f"xq{k}", name=f"xq_sb{k}")
                         for k in range(KCH)]
                for k in range(KCH):
                    nc.sync.dma_start(out=xq_sb[k][:, :],
                                      in_=xq_d[k, :, q0:q0 + QBLK * QT])
                cand_sb = cand_pool.tile([128, QBLK * 64], dt.float32, tag="cand")
                for j in range(QBLK):
                    for f in range(NFC):
                        ps = psum_pool.tile([128, FCH], dt.float32, tag="ps")
                        nc.tensor.matmul(
                            ps[:, :],
                            lhsT=neg1_sb[:, :],
                            rhs=sqy_sb[:, f * FCH:(f + 1) * FCH],
                            start=True,
                            stop=False,
                        )
                        for k in range(KCH):
                            nc.tensor.matmul(
                                ps[:, :],
                                lhsT=xq_sb[k][:, j * QT:(j + 1) * QT],
                                rhs=xs_sb[k][:, f * FCH:(f + 1) * FCH],
                                start=False,
                                stop=(k == KCH - 1),
                            )
                        o = j * 64 + f * 8
                        nc.vector.max(cand_sb[:, o:o + 8], ps[:, :])
                nc.sync.dma_start(out=cand_d[blk], in_=cand_sb[:, :])

    nc.compile()
    return nc


def _get_program():
    if "nc" not in _cache:
        _cache["nc"] = _build_program()
    return _cache["nc"]


def _prep_inputs(X, left, right):
    """X: [N, 512] fp32. Returns (shared xq map entries, per-core xs/sqy)."""
    q_idx = np.concatenate([
        right, np.zeros(TPAD - T_, np.int64),
        left, np.zeros(TPAD - T_, np.int64),
    ])
    Xq = (2.0 * X[q_idx]).astype(np.float32)
    Xq[T_:TPAD] = 0.0
    Xq[TPAD + T_:] = 0.0
    import ml_dtypes

    bf16 = ml_dtypes.bfloat16
    xq_in = np.ascontiguousarray(Xq.T.reshape(KCH, 128, NQ))
    neg1 = np.full((2, QT), -1.0, bf16)

    per_core = []
    for corei in range(NCORES):
        shard = X[corei * NSHARD:(corei + 1) * NSHARD]          # [3750, 512]
        xs = np.zeros((KD, NPAD), np.float32)
        xs[:, :NSHARD] = shard.T
        sqy = np.full(NPAD, PAD_SQY, np.float32)
        sqy[:NSHARD] = (shard.astype(np.float64) ** 2).sum(1).astype(np.float32)
        sqy_hi = sqy.astype(bf16)
        sqy_lo = (sqy - sqy_hi.astype(np.float32)).astype(bf16)
        per_core.append({
            "xq": xq_in,
            "xs": np.ascontiguousarray(xs.reshape(KCH, 128, NPAD)),
            "sqy": np.stack([sqy_hi, sqy_lo]),
            "neg1": neg1,
        })
    return per_core


def _mine_scores(in_maps, trace=False):
    from concourse.bass_utils import run_bass_kernel_spmd

    nc = _get_program()
    try:
        res = run_bass_kernel_spmd(nc, in_maps, list(range(NCORES)), trace=trace)
    except Exception:
        if not trace:
            raise
        res = run_bass_kernel_spmd(nc, in_maps, list(range(NCORES)), trace=False)
    _cache["last_result"] = res
    # per-core cand: [NBLK, 128, QBLK*64] -> [NQ, 64]
    cores = []
    for i in range(NCORES):
        c = res.results[i]["cand"].reshape(NBLK, 128, QBLK, 64)
        cores.append(c.transpose(0, 2, 1, 3).reshape(NQ, 64))
    return np.concatenate(cores, axis=1)                         # [NQ, 512]


def kernel(outlayer, c, train_ill, k):
    k = int(k)
    outlayer = np.asarray(outlayer, np.float32)
    train_ill = np.asarray(train_ill)
    X = np.ascontiguousarray(
        outlayer.transpose(1, 0, 2).reshape(N_, KD)).astype(np.float32)
    left = train_ill[:, 0].astype(np.int64)
    right = train_ill[:, 1].astype(np.int64)

    in_maps = _prep_inputs(X, left, right)
    scores = _mine_scores(in_maps, trace=bool(int(os.environ.get("KNN_TRACE", "0"))))

    # top-(k+1) scores (descending) per query row; row 0 is the self match.
    nkeep = k + 1
    part = np.partition(scores, scores.shape[1] - nkeep, axis=1)[:, -nkeep:]
    top = np.sort(part, axis=1)[:, ::-1]                         # [NQ, k+1]

    X64 = X.astype(np.float64)
    sq = (X64 ** 2).sum(1)                                       # [N]

    s_right = top[:T_]                                           # mining of right idx
    s_left = top[TPAD:TPAD + T_]                                 # mining of left idx

    # B[i, j] = dist(q_i, j-th NN of q_i) = |q_i|^2 - score, self (col 0) dropped
    B2 = sq[right][:, None] - s_right[:, 1:].astype(np.float64)
    B1 = sq[left][:, None] - s_left[:, 1:].astype(np.float64)

    D = ((X64[left] - X64[right]) ** 2).sum(1) + 1.0             # [t]
    L1 = np.maximum(D[:, None] - B1, 0.0)
    L2 = np.maximum(D[:, None] - B2, 0.0)
    loss = (L1.mean() + L2.mean()) / 2.0
    return np.asarray(loss, dtype=np.float32)
